# revision 27
# baseline (speedup 1.0000x reference)
"""Multi-head attention Trainium2 kernel (bs=4, slen=1024, dim=1024, 16 heads).

Sharding: 8 cores = 4 batches x 2 head-groups (8 heads / 512 features each).

v5 (from v4):
  - all on-chip operands fp16 (4x less quantization error than bf16, same
    PE speed); inputs pre-arranged host-side into dense partition-major
    DRAM layouts; input DMA issue spread over the 3 issue-capable queues
    (sync/scalar/gpsimd) in need order, with the first-matmul-critical
    transfers split small and placed first in their queues.
  - ctx matmuls (stationary v, M=65 -> 50% PE column waste) run as fp8e4
    DoubleRow pairs for key tiles 0..5 (two key tiles per instruction,
    2x throughput) and fp16 for key tiles 6..7. rel RMS ~1.6e-2 vs the
    2e-2 gate (fp8 on all 8 tiles would be 1.8e-2; 6/8 keeps margin).
    The softmax denominator (ones-column row 64) is summed from the SAME
    quantized weights, cancelling the common-mode fp8 error.
  - normalize selector matmul fp16; fp16 output store, one DMA per
    (128-row, 512-col) chunk so the final transfer is small.

Pipeline (unchanged from v3):
  PRE : q/k projection for head-pair 0
  A   : scores+exp(hp0) | v projection | q/k proj(hp1)
  B   : scores+exp(hp1) | ctx(hp0) + normalize | q/k proj(hp2)
  C   : scores+exp(hp2) | ctx(hp1) + normalize | q/k proj(hp3)
  D   : scores+exp(hp3) | ctx(hp2) + normalize
  TAIL: ctx(hp3), normalize, out-projection
"""

import numpy as np

BS, SLEN, DIM = 4, 1024, 1024
H, DH = 16, 64
P = 128            # partitions
NB = 512           # matmul free-dim chunk (one PSUM bank of fp32)
FPC = 512          # features per core (8 heads)
DT = DIM // P      # 8 contraction tiles over model dim
FT = FPC // P      # 4 feature tiles per core (== head pairs)
QC = SLEN // NB    # 2 seq chunks
ST = SLEN // P     # 8 seq tiles
HP = 4             # head pairs per core

K8 = 6             # key tiles (of ST) whose ctx runs fp8e4 DoubleRow

_STATE = {}

# set to True by test harness to capture an NTFF profile
TRACE = False
TRACE_KWARGS = {}
LAST_RESULT = None


def _build():
    from contextlib import ExitStack

    import concourse.tile as tile
    from concourse import bacc, mybir

    f32 = mybir.dt.float32
    f16 = mybir.dt.float16
    f8 = mybir.dt.float8e4
    AF = mybir.ActivationFunctionType
    DR = mybir.MatmulPerfMode.DoubleRow

    nc = bacc.Bacc("TRN2", target_bir_lowering=False, debug=False)

    # all inputs pre-arranged host-side into SBUF layout (partition-major)
    xt_d = nc.dram_tensor("xt", [P, QC, DT, NB], f16, kind="ExternalInput")
    wqt_d = nc.dram_tensor("wqt", [P, FT, DT, P], f16, kind="ExternalInput")
    wkt_d = nc.dram_tensor("wkt", [P, FT, DT, P], f16, kind="ExternalInput")
    wvt_d = nc.dram_tensor("wvt", [P, DT, FPC], f16, kind="ExternalInput")
    wot_d = nc.dram_tensor("wot", [P, FT, DIM], f16, kind="ExternalInput")
    qb_d = nc.dram_tensor("qb", [P, FT], f32, kind="ExternalInput")
    kb_d = nc.dram_tensor("kb", [P, FT], f32, kind="ExternalInput")
    negb_d = nc.dram_tensor("negb", [P, ST], f32, kind="ExternalInput")
    sel_d = nc.dram_tensor("sel", [2, P], f16, kind="ExternalInput")
    out_d = nc.dram_tensor("out", [ST, P, QC, NB], f16, kind="ExternalOutput")

    with tile.TileContext(nc) as tc:
        with ExitStack() as ctx:
            consts = ctx.enter_context(tc.tile_pool(name="consts", bufs=1))
            big = ctx.enter_context(tc.tile_pool(name="big", bufs=1))
            wtsp = ctx.enter_context(tc.tile_pool(name="wtsp", bufs=6))
            sm = ctx.enter_context(tc.tile_pool(name="sm", bufs=4))
            psum = ctx.enter_context(tc.tile_pool(name="psum", bufs=1, space="PSUM"))

            # warmup exp on ScalarE (hoists the ACT table load)
            ones_f = consts.tile([1, P], f32)
            nc.vector.memset(ones_f, 1.0)
            warm = consts.tile([1, P], f32)
            nc.scalar.activation(warm, ones_f, AF.Exp, scale=1.0)

            wq_sb = big.tile([P, FT, DT, P], f16, tag="wq")
            wk_sb = big.tile([P, FT, DT, P], f16, tag="wk")
            wvt_sb = big.tile([P, DT, FPC], f16, tag="wv")
            wot_sb = big.tile([P, FT, DIM], f16, tag="wo")
            xt_a = big.tile([P, QC, DT // 2, NB], f16, tag="xa")
            xt_b = big.tile([P, QC, DT // 2, NB], f16, tag="xb")
            qb_sb = consts.tile([P, FT], f32)
            kb_sb = consts.tile([P, FT], f32)
            negb_sb = consts.tile([P, ST], f32)
            sel_sb = consts.tile([2, P], f16)
            selB_sb = consts.tile([1, P], f16)

            def xts(t, qc):
                xh = xt_a if t < DT // 2 else xt_b
                return xh[:, qc, t % (DT // 2), :]

            def wcol(eng, w_d, w_sb, ft, th=None):
                ts = slice(0, DT) if th is None else slice(
                    th * (DT // 2), (th + 1) * (DT // 2))
                eng.dma_start(w_sb[:, ft, ts, :], w_d[:, ft, ts, :])

            def xchunk(eng, half, qc, ph=None):
                """ph splits the 128 partitions in half: one dma_start's
                descriptors land on a single engine's DMA-queue pair, so a
                critical transfer is parallelized by splitting it across
                two ENGINES, not by more descriptors."""
                xh = xt_a if half == 0 else xt_b
                ts = slice(half * (DT // 2), (half + 1) * (DT // 2))
                ps = slice(0, P) if ph is None else slice(
                    ph * (P // 2), (ph + 1) * (P // 2))
                eng.dma_start(xh[ps, qc, :, :], xt_d[ps, qc, ts, :])

            # ---- input DMA issue: 3 queues, need order; the loads the
            # first q-projection group needs are split small + first ----
            xchunk(nc.sync, 0, 0)
            wcol(nc.scalar, wqt_d, wq_sb, 0, th=0)
            wcol(nc.scalar, wqt_d, wq_sb, 0, th=1)
            xchunk(nc.sync, 1, 0)
            wcol(nc.scalar, wkt_d, wk_sb, 0, th=0)
            wcol(nc.scalar, wkt_d, wk_sb, 0, th=1)
            nc.gpsimd.dma_start(qb_sb, qb_d[:])
            nc.gpsimd.dma_start(kb_sb, kb_d[:])
            xchunk(nc.gpsimd, 0, 1)
            xchunk(nc.gpsimd, 1, 1)
            nc.gpsimd.dma_start(negb_sb, negb_d[:])
            nc.gpsimd.dma_start(wvt_sb, wvt_d[:])
            wcol(nc.scalar, wqt_d, wq_sb, 1)
            wcol(nc.sync, wkt_d, wk_sb, 1)
            wcol(nc.sync, wqt_d, wq_sb, 2)
            wcol(nc.gpsimd, wkt_d, wk_sb, 2)
            nc.gpsimd.dma_start(sel_sb, sel_d[:])
            nc.gpsimd.dma_start(selB_sb, sel_d[1:2, :])
            wcol(nc.scalar, wqt_d, wq_sb, 3)
            wcol(nc.sync, wkt_d, wk_sb, 3)
            nc.scalar.dma_start(wot_sb, wot_d[:])

            # ---- PE p-state warm-up on local data during input DMA ----
            wrm = consts.tile([P, P], f16, name="wrm")
            nc.vector.memset(wrm, 0.0)
            for i in range(38):
                ps_w = psum.tile([P, 2, NB], f32, tag="s", bufs=2,
                                 name="ps_w")
                nc.tensor.matmul(ps_w[:, 0, 0:P], lhsT=wrm, rhs=wrm)

            # ---- persistent activations ----
            qT_sb = big.tile([P, FT, SLEN], f16, tag="qT")   # [f%128, ft, seq]
            kT_sb = big.tile([P, FT, SLEN], f16, tag="kT")
            # v8 rows padded to DH+2 (66): dual-fp8 weight loads need even
            # column counts/offsets. col DH = ones (denominator), col DH+1
            # = zeros (junk psum row 65, never read).
            v8_sb = big.tile([P, K8, HP * 2, DH + 2], f8, tag="v8")
            if K8 < ST:
                v16_sb = big.tile([P, ST - K8, HP * 2, DH + 1], f16,
                                  tag="v16", name="v16_sb")
            else:
                v16_sb = None
            ctall = big.tile([P, HP * QC, NB], f16, tag="ct")  # unnormalized
            ctxn_sb = big.tile([P, HP, SLEN], f16, tag="cn")   # normalized
            vones_f = consts.tile([P, ST, HP * 2, 1], f32)
            nc.vector.memset(vones_f, 1.0)
            nc.vector.tensor_copy(v8_sb[:, :, :, DH:DH + 1],
                                  vones_f[:, 0:K8, :, :])
            nc.vector.memset(v8_sb[:, :, :, DH + 1:DH + 2], 0.0)
            if v16_sb is not None:
                nc.vector.tensor_copy(v16_sb[:, :, :, DH:DH + 1],
                                      vones_f[:, K8:ST, :, :])

            rca = {}  # (hp, qc) -> [2, NB] f16 reciprocal-denominator tile

            # ---- emission helpers (pipeline stages) ----
            def qkproj_group(ft, qc, which):
                """One 8-deep accumulation group of q or k projection,
                drained on VectorE with the bias add."""
                sl = slice(qc * NB, (qc + 1) * NB)
                w_sb = wq_sb if which == 0 else wk_sb
                dst = qT_sb if which == 0 else kT_sb
                b_sb = qb_sb if which == 0 else kb_sb
                ps = psum.tile([P, NB], f32, tag="aux", bufs=2, name="ps_p")
                for t in range(DT):
                    nc.tensor.matmul(
                        ps, lhsT=w_sb[:, ft, t, :], rhs=xts(t, qc),
                        start=(t == 0), stop=(t == DT - 1))
                nc.vector.tensor_scalar_add(
                    dst[:, ft, sl], ps, b_sb[:, ft:ft + 1])

            def vproj_group(st):
                """v projection for one seq tile (all 8 heads), drained on
                VectorE (fp8 store for the DoubleRow key tiles). v_b is NOT
                added: softmax weights sum to 1, so its contribution is the
                constant v_b @ out_w.T, folded into out_b on the host."""
                qc, sub = divmod(st, ST // QC)
                ps_v = psum.tile([P, NB], f32, tag="aux", bufs=2, name="ps_v")
                for t in range(DT):
                    nc.tensor.matmul(
                        ps_v, lhsT=xts(t, qc)[:, sub * P:(sub + 1) * P],
                        rhs=wvt_sb[:, t, :],
                        start=(t == 0), stop=(t == DT - 1))
                dst = (v8_sb[:, st, :, 0:DH] if st < K8
                       else v16_sb[:, st - K8, :, 0:DH])
                nc.vector.tensor_copy(
                    dst, ps_v.rearrange("p (h e) -> p h e", h=HP * 2))

            def scores_pair(hp, qc, kt, wts8, wts16):
                """scoresT for both heads of pair hp into one 2-bank PSUM
                tile; single merged Exp on ScalarE (scale=1/8, mask bias),
                written fp8 for DoubleRow key tiles, fp16 otherwise."""
                sl = slice(qc * NB, (qc + 1) * NB)
                ksl = slice(kt * P, (kt + 1) * P)
                ps = psum.tile([P, 2, NB], f32, tag="s", bufs=2, name="ps_s")
                nc.tensor.matmul(
                    ps[:, 0, :], lhsT=kT_sb[0:DH, hp, ksl],
                    rhs=qT_sb[0:DH, hp, sl], tile_position=(0, 0))
                nc.tensor.matmul(
                    ps[:, 1, :], lhsT=kT_sb[DH:P, hp, ksl],
                    rhs=qT_sb[DH:P, hp, sl], tile_position=(DH, 0))
                dst = (wts8[:, kt, :, :] if kt < K8
                       else wts16[:, kt - K8, :, :])
                nc.scalar.activation(
                    dst, ps[:, :, :], AF.Exp,
                    bias=negb_sb[:, kt:kt + 1], scale=0.125)

            def ctx_pair(hp, qc, k2, pcA, pcB, wts8, wts16):
                """ctxT accumulation for key tiles 2*k2, 2*k2+1: one fp8
                DoubleRow matmul per head below K8, fp16 singles above."""
                for a, pc in ((0, pcA), (1, pcB)):
                    if 2 * k2 + 1 < K8:
                        ks = slice(2 * k2, 2 * k2 + 2)
                        nc.tensor.matmul(
                            pc, lhsT=v8_sb[:, ks, 2 * hp + a, :],
                            rhs=wts8[:, ks, a, :], perf_mode=DR,
                            start=(k2 == 0), stop=(2 * k2 + 2 == ST),
                            skip_group_check=True)
                    else:
                        for kt in (2 * k2, 2 * k2 + 1):
                            nc.tensor.matmul(
                                pc[0:DH + 1, :],
                                lhsT=v16_sb[:, kt - K8, 2 * hp + a, :],
                                rhs=wts16[:, kt - K8, a, :],
                                start=False, stop=(kt == ST - 1),
                                skip_group_check=True)

            def ctx_drain(hp, qc, pcA, pcB, use_scalar=False):
                """ctxT + denominator rows out of PSUM, then the fast
                approximate reciprocal of the [2,512] denominator."""
                j = hp * QC + qc
                from concourse.dve_ops import (
                    RECIP_APPROX_FAST_CONSTS, RECIPROCAL_APPROX_FAST)
                c = RECIP_APPROX_FAST_CONSTS
                rs = []
                for a, pc in ((0, pcA), (1, pcB)):
                    dtmp = sm.tile([1, NB], f32, tag="dtmp", bufs=4,
                                   name="dtmp")
                    if use_scalar and a == 1:
                        nc.scalar.copy(
                            ctall[a * DH:(a + 1) * DH, j, :], pc[0:DH, :])
                        nc.scalar.copy(dtmp, pc[DH:DH + 1, :])
                    else:
                        nc.vector.tensor_copy(
                            ctall[a * DH:(a + 1) * DH, j, :], pc[0:DH, :])
                        nc.vector.tensor_copy(dtmp, pc[DH:DH + 1, :])
                    # per-head reciprocal straight from the staged row: no
                    # SBUF->SBUF DMA round-trip in the normalize chain
                    r1 = sm.tile([1, NB], f16, tag="rca2", bufs=8,
                                 name="rca1")
                    nc.vector._custom_dve(
                        RECIPROCAL_APPROX_FAST, out=r1, in0=dtmp,
                        s0=c["s0"], s1=c["s1"], imm2=c["imm2"])
                    rs.append(r1)
                rca[(hp, qc)] = rs

            def normalize(hp, qc):
                j = hp * QC + qc
                sl = slice(qc * NB, (qc + 1) * NB)
                pb = psum.tile([P, NB], f32, tag="aux", bufs=2, name="pb")
                rA, rB = rca[(hp, qc)]
                nc.tensor.matmul(pb, lhsT=sel_sb[0:1, :], rhs=rA,
                                 start=True, stop=False)
                nc.tensor.matmul(pb, lhsT=selB_sb, rhs=rB,
                                 start=False, stop=True)
                nc.vector.tensor_mul(ctxn_sb[:, hp, sl], ctall[:, j, :], pb)

            def outproj_qt(qt):
                """Out-projection for one 128-row seq tile (PSUM out is
                capped at one bank per matmul, so two 512-wide groups);
                drains alternate between ScalarE and VectorE, one output
                DMA per 512-col chunk so the final transfer is small."""
                po = psum.tile([P, 2, NB], f32, tag="s", bufs=2, name="po_s")
                ob2 = sm.tile([P, 2, NB], f16, tag="outsb", bufs=3,
                              name="ob2")
                for jc in range(QC):
                    for ft in range(FT):
                        nc.tensor.matmul(
                            po[:, jc, :],
                            lhsT=ctxn_sb[:, ft, qt * P:(qt + 1) * P],
                            rhs=wot_sb[:, ft, jc * NB:(jc + 1) * NB],
                            start=(ft == 0), stop=(ft == FT - 1))
                    if (qt + jc) % 2 == 0:
                        nc.scalar.copy(ob2[:, jc, :], po[:, jc, :])
                    else:
                        nc.vector.tensor_copy(ob2[:, jc, :], po[:, jc, :])
                    oeng = (nc.scalar if qt == ST - 1
                            else nc.sync if qt % 2 == 0 else nc.gpsimd)
                    oeng.dma_start(out_d[qt, :, jc, :], ob2[:, jc, :])

            def wts_tiles(name):
                w8 = wtsp.tile([P, K8, 2, NB], f8, tag="wts8",
                               name=name + "_8")
                if K8 < ST:
                    w16 = wtsp.tile([P, ST - K8, 2, NB], f16, tag="wts16",
                                    name=name + "_16")
                else:
                    w16 = None
                return (w8, w16)

            def pe_filler(n):
                for _ in range(n):
                    nc.tensor.ldweights(wrm[:, 0:P])

            # ---- PRE: q/k projection for head pair 0 ----
            wts_cur = {}
            pc_cur = {}
            with nc.named_scope("pre"):
                for qc in range(QC):
                    for which in range(2):
                        qkproj_group(0, qc, which)
                        pe_filler(8)
            for step in range(HP):        # step = hp being scored
                with nc.named_scope(f"step{step}"):
                    for qc in range(QC):
                        if (step, qc) not in wts_cur:
                            wts_cur[(step, qc)] = wts_tiles(
                                f"wts_{step}_{qc}")
                    for kt in range(ST):
                        scores_pair(step, 0, kt, *wts_cur[(step, 0)])
                        scores_pair(step, 1, kt, *wts_cur[(step, 1)])
                        if step == 0:
                            vproj_group(kt)
                        else:
                            hp = step - 1
                            qc = 0 if kt < 4 else 1
                            k2 = kt % 4
                            if k2 == 0:
                                pc_cur[0] = psum.tile(
                                    [DH + 2, NB], f32, tag="cA", bufs=1,
                                    name="pcA")
                                pc_cur[1] = psum.tile(
                                    [DH + 2, NB], f32, tag="cB", bufs=1,
                                    name="pcB")
                            ctx_pair(hp, qc, k2, pc_cur[0], pc_cur[1],
                                     *wts_cur[(hp, qc)])
                            if k2 == 3:
                                ctx_drain(hp, qc, pc_cur[0], pc_cur[1])
                        if kt == 6 and step >= 1:
                            normalize(step - 1, 0)
                        if kt == 1 and step >= 2:
                            normalize(step - 2, 1)
                        # q/k proj for head pair step+1: 4 groups of 8.
                        # The last group of pair 3 (keys 512:1023) moves into
                        # step D, which is otherwise scalar-bound; scores of
                        # key tiles >= 4 only start at iteration 4.
                        if step < HP - 1 and kt % 2 == 0:
                            g = kt // 2
                            if not (step == HP - 2 and g == 3):
                                qkproj_group(step + 1, g // 2, g % 2)
                        if step == HP - 1 and kt == 0:
                            qkproj_group(HP - 1, 1, 1)
                        if step == HP - 1 and kt >= 2:
                            pe_filler(5)

            # ---- tail: ctx(hp3) interleaved with out-projection ----
            with nc.named_scope("tail"):
                hp = HP - 1
                pcA = psum.tile([DH + 2, NB], f32, tag="cA", bufs=1,
                                name="pcA_t0")
                pcB = psum.tile([DH + 2, NB], f32, tag="cB", bufs=1,
                                name="pcB_t0")
                for k2 in range(4):
                    ctx_pair(hp, 0, k2, pcA, pcB, *wts_cur[(hp, 0)])
                pe_filler(4)
                normalize(HP - 2, 1)   # pending from step D
                ctx_drain(hp, 0, pcA, pcB, use_scalar=True)
                pe_filler(4)
                pcA = psum.tile([DH + 2, NB], f32, tag="cA", bufs=1,
                                name="pcA_t1")
                pcB = psum.tile([DH + 2, NB], f32, tag="cB", bufs=1,
                                name="pcB_t1")
                ctx_pair(hp, 1, 0, pcA, pcB, *wts_cur[(hp, 1)])
                ctx_pair(hp, 1, 1, pcA, pcB, *wts_cur[(hp, 1)])
                pe_filler(4)
                normalize(hp, 0)
                outproj_qt(0)
                pe_filler(3)
                ctx_pair(hp, 1, 2, pcA, pcB, *wts_cur[(hp, 1)])
                ctx_pair(hp, 1, 3, pcA, pcB, *wts_cur[(hp, 1)])
                pe_filler(3)
                outproj_qt(1)
                ctx_drain(hp, 1, pcA, pcB, use_scalar=True)
                outproj_qt(2)
                outproj_qt(3)
                normalize(hp, 1)
                for qt in range(ST // QC, ST):
                    outproj_qt(qt)

    nc.compile()
    return nc


def _get_nc():
    if "nc" not in _STATE:
        _STATE["nc"] = _build()
    return _STATE["nc"]


def _sel_const():
    sel = np.zeros((2, P), np.float16)
    sel[0, 0:DH] = 1.0
    sel[1, DH:P] = 1.0
    return sel


def _in_maps(x, mask, q_w, q_b, k_w, k_b, v_w, v_b, out_w):
    f = np.float32
    h = np.float16
    maps = []
    for c in range(8):
        b, g = divmod(c, 2)
        fs = slice(g * FPC, (g + 1) * FPC)
        # x.T: row d = t*P + p; -> [p, qc, t, nb]
        xt = (x[b].T.astype(h).reshape(DT, P, QC, NB)
              .transpose(1, 2, 0, 3).copy())
        # w.T: [dim, fpc], row d = t*P + p; -> [p, ft, t, col]
        wq = (q_w[fs, :].T.astype(h).reshape(DT, P, FT, P)
              .transpose(1, 2, 0, 3).copy())
        wk = (k_w[fs, :].T.astype(h).reshape(DT, P, FT, P)
              .transpose(1, 2, 0, 3).copy())
        wv = (v_w[fs, :].T.astype(h).reshape(DT, P, FPC)
              .transpose(1, 0, 2).copy())
        # out_w[:, fs].T: [fpc, dim], row f = ft*P + p; -> [p, ft, dim]
        wo = (out_w[:, fs].T.astype(h).reshape(FT, P, DIM)
              .transpose(1, 0, 2).copy())
        maps.append({
            "xt": xt, "wqt": wq, "wkt": wk, "wvt": wv, "wot": wo,
            "qb": np.ascontiguousarray(q_b[fs].astype(f).reshape(FT, P).T),
            "kb": np.ascontiguousarray(k_b[fs].astype(f).reshape(FT, P).T),
            "negb": np.ascontiguousarray(
                np.where(mask[b] == 0, f(-30000.0), f(0.0)).astype(f)
                .reshape(ST, P).T),
            "sel": _sel_const(),
        })
    return maps


def kernel(x, mask, q_w, q_b, k_w, k_b, v_w, v_b, out_w, out_b):
    global LAST_RESULT
    from concourse import bass_utils

    x = np.asarray(x, np.float32)
    mask = np.asarray(mask)
    nc = _get_nc()
    maps = _in_maps(x, mask, np.asarray(q_w, np.float32),
                    np.asarray(q_b, np.float32), np.asarray(k_w, np.float32),
                    np.asarray(k_b, np.float32), np.asarray(v_w, np.float32),
                    np.asarray(v_b, np.float32), np.asarray(out_w, np.float32))
    res = bass_utils.run_bass_kernel_spmd(
        nc, maps, core_ids=list(range(8)), trace=TRACE,
        trace_kwargs=TRACE_KWARGS)
    LAST_RESULT = res
    # v_b's contribution to the output is the constant v_b @ out_w.T
    # (softmax weights sum to 1), folded into the output bias here.
    out_b = np.asarray(out_b, np.float32) + (
        np.asarray(v_b, np.float32) @ np.asarray(out_w, np.float32).T)
    full = np.empty((BS, SLEN, DIM), np.float32)
    for b in range(BS):
        oa = res.results[2 * b]["out"].astype(np.float32).reshape(SLEN, DIM)
        ob = res.results[2 * b + 1]["out"].astype(np.float32).reshape(SLEN, DIM)
        full[b] = oa + ob + out_b
    return full
